# revision 1
# baseline (speedup 1.0000x reference)
"""NT-Xent loss kernel for Trainium2 (8 NeuronCores, Bass/Tile).

Symmetric "wedge" decomposition: z = concat(z1, z2) is cast to bf16
(the matmul runs in bf16 anyway) and each core receives
np.roll(z, -1024*c, axis=0), so core-local rows [0, 1024) are its
assigned rows.  Each unordered pair {a, b} of the 2Nx2N similarity
matrix is computed exactly once: a core computes columns [0, 4096+w_i)
of its row block, where columns [0, 1024) are the (symmetric) diagonal
block and the triangular band beyond 4096 keeps only distances
d < 4096 (the rest is masked to -1e6 before exp).  Each exp credits
its own row via the ACT accum_out row sums AND its column's row via
all-ones column-sum matmuls chained through PSUM has_written
accumulation.  The host un-rotates and sums row/column partials from
all cores, adds exp(10*pos) for the masked distance-4096 positives in
fp64, and takes log + mean.

Per core:
  - Row norms on DVE (bn_stats fields combined directly), rsqrt by
    Newton iteration (DVE only, so the ACT exp table loads once).
  - Normalize in natural layout (bf16 4x DVE), PE-transpose into znT.
  - 2.5 column slabs; per row tile: bf16 matmuls (K=256, N=512) into a
    [128,2048] PSUM tile, diagonal/boundary masks added in PSUM, the
    positive-pair diagonal extracted on DVE, then one ACT Exp(scale=10)
    with accum_out writing E (bf16) for the column-sum chains.
  - Slab prologues and column-sum chains are emission-interleaved into
    the running batches so ACT stays fed.
  - Outputs: [128, 16] row sums + positive dots, [1, 4096] column sums.
"""

import sys

if "/opt/trn_rl_repo" not in sys.path:
    sys.path.insert(0, "/opt/trn_rl_repo")

import numpy as np
import ml_dtypes

import concourse.bacc as bacc
import concourse.mybir as mybir
import concourse.tile as tile
from concourse.masks import make_identity

P = 128
D = 256
M = 8192            # 2N rows
NCORES = 8
NT = M // P         # 64 row tiles
IT = (M // NCORES) // P   # 8 row tiles owned per core
NSL = 4             # slabs of 2048 rows (= one 2048-wide column batch)
TPS = NT // NSL     # 16 row tiles per slab
TEMP_INV = 10.0     # 1 / temperature
F32 = mybir.dt.float32
BF16 = mybir.dt.bfloat16
FP8 = mybir.dt.float8e5
CHUNK = 2048
NCH = M // CHUNK    # 4 column batches
NSUB = CHUNK // 512

_nc_cache = None


def _build():
    nc = bacc.Bacc(None, target_bir_lowering=False)
    z = nc.dram_tensor("z", [M, D], BF16, kind="ExternalInput")
    mtri_in = nc.dram_tensor("mtri", [P, P], F32, kind="ExternalInput")
    out = nc.dram_tensor("out", [P, 2 * IT], F32, kind="ExternalOutput")
    csout = nc.dram_tensor("csout", [1, 4096], F32, kind="ExternalOutput")

    AF = mybir.ActivationFunctionType
    ALU = mybir.AluOpType

    with (
        tile.TileContext(nc) as tc,
        tc.tile_pool(name="big", bufs=1) as big,
        tc.tile_pool(name="small", bufs=1) as small,
        tc.tile_pool(name="zpool", bufs=2) as zpool,
        tc.tile_pool(name="psp", bufs=2, space="PSUM") as psp,
    ):
        # per-slab tiles (separate handles so the tile-granular dependency
        # tracker never serializes one slab's transpose behind another
        # slab's normalize)
        znns = [big.tile([P, TPS, D], BF16, name=f"znn_{s}") for s in range(NSL)]
        znTs = [big.tile([P, 2, CHUNK], BF16, name=f"znT_{s}") for s in range(NSL)]
        Es = [big.tile([P, IT, CHUNK], BF16, name=f"E_{b}") for b in range(2)]
        cs_sb = big.tile([P, 4096], F32)     # colsums for local cols [1024,5120)
        pos_dead = big.tile([P, P], F32)
        stats = small.tile([P, NT, 6], F32)
        ss = small.tile([P, NT], F32)        # row norm^2
        rn = small.tile([P, NT], F32)        # 1/norm (natural layout)
        nt1 = small.tile([P, NT], F32)       # newton scratch
        acc = small.tile([P, IT, 4], F32)
        pp = small.tile([P, IT], F32)        # positive-pair dots
        identb = small.tile([P, P], BF16)
        onesb = small.tile([P, P], BF16)
        mtri = small.tile([P, P], F32)       # -1e6 where col >= row
        identf = small.tile([P, P], F32)
        maskd = small.tile([P, P], F32)      # -1e6 on the diagonal
        make_identity(nc, identb)
        make_identity(nc, identf)
        nc.vector.tensor_scalar_mul(maskd, identf, -1.0e6)
        nc.vector.memset(onesb, 1.0)
        nc.sync.dma_start(out=mtri, in_=mtri_in[:, :])

        zv = z.rearrange("(t p) d -> p t d", p=P)

        def bwidth(c, i):
            # batch 2 is the triangular distance band [4096, 4096+128(i+1))
            # rounded up to 512 columns; batches 0/1 are full width
            return 512 * ((i + 4) // 4) if c == 2 else CHUNK

        def emit_main_batch(c, i0=0, i1=IT, n0=0, n1=None, aslot=None):
            for i in range(i0, i1):
                w = bwidth(c, i)
                nn1 = w // 512 if n1 is None else n1
                asl = c if aslot is None else aslot
                ps = psp.tile(
                    [P, (nn1 - n0) * 512], F32, tag="ps",
                    name=f"ps_{i}_{c}_{n0}",
                )
                for k in range(2):
                    for n in range(n0, nn1):
                        nc.tensor.matmul(
                            ps[:, (n - n0) * 512 : (n - n0 + 1) * 512],
                            lhsT=znTs[0][:, k, i * P : (i + 1) * P],
                            rhs=znTs[c][:, k, n * 512 : (n + 1) * 512],
                            start=(k == 0),
                            stop=(k == 1),
                        )
                if c == 0 and n0 == 0:
                    # self-similarity -> exp(...) == 0 (all diagonal
                    # subtiles sit in columns [0, 1024))
                    nc.vector.tensor_add(
                        ps[:, i * P : (i + 1) * P],
                        ps[:, i * P : (i + 1) * P],
                        maskd,
                    )
                if c == 2:
                    # positive-pair dots live on the diagonal of the
                    # subtile at column 4096 + 128*i; extract BEFORE the
                    # boundary masks kill them (host adds exp back)
                    nc.vector.tensor_mul(
                        pos_dead, ps[:, i * P : (i + 1) * P], identf
                    )
                    nc.vector.reduce_sum(
                        pp[:, i : i + 1], pos_dead, axis=mybir.AxisListType.X
                    )
                    # mask d >= 4096: upper-incl-diag of the boundary
                    # subtile plus everything to its right (those pairs
                    # are owned by the mirror cores)
                    nc.vector.tensor_add(
                        ps[:, i * P : (i + 1) * P],
                        ps[:, i * P : (i + 1) * P],
                        mtri,
                    )
                    if w > (i + 1) * P:
                        nc.vector.tensor_scalar_add(
                            ps[:, (i + 1) * P : w],
                            ps[:, (i + 1) * P : w],
                            -1.0e6,
                        )
                nc.scalar.activation(
                    out=Es[c % 2][:, i, n0 * 512 : nn1 * 512],
                    in_=ps[:, 0 : (nn1 - n0) * 512],
                    func=AF.Exp,
                    scale=TEMP_INV,
                    accum_out=acc[:, i, asl : asl + 1],
                )

        def emit_colsums(c, s0=0, s1=None):
            # credit each computed element's exp to its column's own row
            # via all-ones matmuls chained through PSUM has_written
            # accumulation (batch 0 skips the diagonal block's columns,
            # which are complete in-row already)
            lo, hi = {0: (1024, 2048), 1: (0, 2048), 2: (0, 1024)}[c]
            for off in list(range(lo, hi, 512))[s0:s1]:
                rts = [i for i in range(IT) if bwidth(c, i) >= off + 512]
                cps = psp.tile([P, 512], F32, tag="ps", name=f"cs_{c}_{off}")
                for x, i in enumerate(rts):
                    nc.tensor.matmul(
                        cps[:, :],
                        lhsT=onesb,
                        rhs=Es[c % 2][:, i, off : off + 512],
                        start=(x == 0),
                        stop=(x == len(rts) - 1),
                    )
                nc.vector.tensor_copy(
                    out=cs_sb[:, c * CHUNK + off - 1024 : c * CHUNK + off - 512],
                    in_=cps[:, :],
                )

        def prologue(s, t0=0, t1=TPS):
            ntile = t1 - t0
            ts = slice(s * TPS + t0, s * TPS + t1)
            zg = zpool.tile([P, ntile, D], BF16, tag="zg", name=f"zg_{s}")
            (nc.sync if s % 2 == 0 else nc.gpsimd).dma_start(
                out=zg, in_=zv[:, ts, :]
            )
            # norms: norm^2 = D * (var + mean^2), on DVE
            for j in range(ntile):
                nc.vector.bn_stats(stats[:, s * TPS + t0 + j, :], zg[:, j, :])
            # norm^2 directly from bn_stats halves:
            #   [cnt, mean_a, M2_a, cnt, mean_b, M2_b] per tile
            #   norm^2 = M2_a + M2_b + 128*(mean_a^2 + mean_b^2)
            nc.vector.tensor_mul(
                ss[:, ts], stats[:, ts, 1], stats[:, ts, 1]
            )
            nc.vector.tensor_mul(
                nt1[:, ts], stats[:, ts, 4], stats[:, ts, 4]
            )
            nc.vector.tensor_add(ss[:, ts], ss[:, ts], nt1[:, ts])
            nc.vector.tensor_scalar_mul(ss[:, ts], ss[:, ts], float(P))
            nc.vector.tensor_add(ss[:, ts], ss[:, ts], stats[:, ts, 2])
            nc.vector.tensor_add(ss[:, ts], ss[:, ts], stats[:, ts, 5])
            # rn = 1/sqrt(ss) by Newton on DVE (keeps ACT exp-table
            # resident).  ss = |z_row|^2 is chi^2(256)-concentrated in
            # [180, 340], so y0 = 1/16 converges to <1e-5 in 3 steps.
            nc.vector.memset(rn[:, ts], 0.0625)
            for _ in range(3):
                nc.vector.tensor_mul(nt1[:, ts], rn[:, ts], rn[:, ts])
                nc.vector.tensor_mul(nt1[:, ts], nt1[:, ts], ss[:, ts])
                nc.vector.tensor_scalar(
                    out=nt1[:, ts], in0=nt1[:, ts],
                    scalar1=-0.5, scalar2=1.5,
                    op0=ALU.mult, op1=ALU.add,
                )
                nc.vector.tensor_mul(rn[:, ts], rn[:, ts], nt1[:, ts])
            # normalize in natural layout (bf16 in/out -> DVE 4x mode)
            for j in range(ntile):
                t = s * TPS + t0 + j
                nc.vector.tensor_scalar_mul(
                    znns[s][:, t0 + j, :], zg[:, j, :], rn[:, t : t + 1]
                )
            # PE-transpose the slab into znT (32 [128,128] blocks)
            pt = psp.tile([P, 2, TPS, P], BF16, tag="ps", name=f"pt_{s}_{t0}")
            for j in range(ntile):
                for k in range(2):
                    nc.tensor.transpose(
                        pt[:, k, j, :],
                        znns[s][:, t0 + j, k * P : (k + 1) * P],
                        identb,
                    )
            for k in range(2):
                nc.vector.tensor_copy(
                    out=znTs[s][:, k, t0 * P : t1 * P],
                    in_=pt[:, k, 0:ntile].rearrange("p j c -> p (j c)"),
                )

        # pipeline: batch s starts as soon as slab s is transposed; slab
        # s+1's prologue+transposes are emitted after batch s's first two
        # chunks so they complete well before batch s+1 needs them
        # slab 0 in two halves so the first (1024-wide) half of batch 0
        # starts as early as possible
        prologue(0, 0, TPS // 2)
        emit_main_batch(0, 0, IT, 0, 2, aslot=0)   # cols [0, 1024)
        prologue(0, TPS // 2, TPS)
        emit_main_batch(0, 0, 2, 2, 4, aslot=3)    # cols [1024, 2048)
        prologue(1)
        emit_main_batch(0, 2, IT, 2, 4, aslot=3)
        emit_main_batch(1, 0, 2)
        emit_colsums(0, 0, 1)
        prologue(2, 0, TPS // 2)
        emit_main_batch(1, 2, 4)
        emit_colsums(0, 1, 2)
        emit_main_batch(1, 4, IT)
        emit_main_batch(2, 0, 2)
        emit_colsums(1, 0, 2)
        emit_main_batch(2, 2, 4)
        emit_colsums(1, 2, 4)
        emit_main_batch(2, 4, IT)
        emit_colsums(2)

        # ---- tail: denominators and output ----
        outs = small.tile([P, 2 * IT], F32)
        nc.vector.reduce_sum(
            outs[:, 0:IT], acc, axis=mybir.AxisListType.X
        )
        nc.vector.tensor_copy(out=outs[:, IT : 2 * IT], in_=pp)
        nc.sync.dma_start(out=out[:, :], in_=outs)
        nc.sync.dma_start(out=csout[0:1, :], in_=cs_sb[0:1, :])

    nc.finalize()
    return nc


def _get_nc():
    global _nc_cache
    if _nc_cache is None:
        _nc_cache = _build()
    return _nc_cache


def _run_cores(z: np.ndarray, trace: bool = False):
    """Run the SPMD kernel on 8 cores. z is [M, D] bf16."""
    from concourse.bass_utils import run_bass_kernel_spmd

    nc = _get_nc()
    rows_per_core = M // NCORES
    mtri = np.where(
        np.arange(P)[None, :] >= np.arange(P)[:, None], -1.0e6, 0.0
    ).astype(np.float32)
    in_maps = [
        {
            "z": np.ascontiguousarray(np.roll(z, -rows_per_core * c, axis=0)),
            "mtri": mtri,
        }
        for c in range(NCORES)
    ]
    res = run_bass_kernel_spmd(
        nc, in_maps, core_ids=list(range(NCORES)), trace=trace
    )
    return res


def kernel(z1: np.ndarray, z2: np.ndarray) -> np.ndarray:
    z = np.concatenate(
        [np.asarray(z1, np.float32), np.asarray(z2, np.float32)], axis=0
    ).astype(ml_dtypes.bfloat16)
    res = _run_cores(z)
    total = np.zeros(M, np.float64)
    pos_sum = 0.0
    for c, r in enumerate(res.results):
        parts = np.asarray(r["out"]).astype(np.float64)
        cs = np.asarray(r["csout"]).astype(np.float64)[0]
        rowsum = parts[:, :IT]        # [128, 8]: local row t*128+p
        pos = parts[:, IT:]
        base = 1024 * c
        for t in range(IT):
            g = (base + t * P + np.arange(P)) % M
            # own-wedge row sums plus the masked positive pair, exp'd on host
            total[g] += rowsum[:, t] + np.exp(TEMP_INV * pos[:, t])
            pos_sum += pos[:, t].sum()
        # colsums credit local columns [1024, 5120)
        g = (base + 1024 + np.arange(4096)) % M
        np.add.at(total, g, cs)
    lse_sum = np.log(total).sum()
    return np.float32((lse_sum - TEMP_INV * pos_sum) / M)



# revision 20
# speedup vs baseline: 1.6774x; 1.6774x over previous
"""NT-Xent loss kernel for Trainium2 (8 NeuronCores, Bass/Tile).

Wedge decomposition with host-side normalization and fp8 matmuls.

Host: z = concat(z1, z2), zn = z/||z||, q = fp8e4(16*zn); each core c
receives qT rolled by -1024*c, pre-transposed as [128, 2, 5120]
(partition = d mod 128, dim1 = d half, dim2 = column).  Positive-pair
dots are computed on host in fp64.

Device (per core, rows [0,1024) local = 8 row tiles of 128):
  sim' = 256*cos via fp8e4 DoubleRow matmuls (K=256 in one pass).
  Columns [0,4096) = full block, [4096,4096+128(i+1)) = triangular
  band (keeps distance < 4096; boundary subtile mtri-masked).
  exp handled by two engines in parallel:
   - DVE path (cols [0,1024) of c0 + [0,512) of c1): one
     scalar_tensor_tensor converts PSUM logits straight to fp8e5 BITS
     (Schraudolph: i8 = sat(round(psum*s1 + master))), masking the
     self-sim diagonal via the `master` window; tensor_tensor_reduce
     over fold-in-half yields row sums.
   - ACT path (the rest): Exp activation with accum_out row sums,
     fp8e5 E output.
  Column sums credit mirror rows via fp8e5 DoubleRow ones-matmuls
  chained over row-tile pairs, packed 4 chunks per PSUM bank on
  32-partition groups.
Host un-rotates, adds exp(10*pos) in fp64, takes log + mean.
"""

import sys

if "/opt/trn_rl_repo" not in sys.path:
    sys.path.insert(0, "/opt/trn_rl_repo")

import numpy as np
import ml_dtypes

import concourse.bacc as bacc
import concourse.mybir as mybir
import concourse.tile as tile

P = 128
D = 256
M = 8192
N2 = 4096
NCORES = 8
IT = 8               # row tiles per core
WCOL = 5120          # columns of znT each core needs
TEMP_INV = 10.0
QS = 16.0            # host quantization scale: q = fp8(QS * zn)
SC = TEMP_INV / (QS * QS)        # ACT exp scale: exp(SC * sim')
LN2 = float(np.log(2.0))
S1D = SC * 128.0 / LN2           # DVE bf16-schraudolph slope
S2D = 16248.64                   # bias (incl. mean-error calibration)
MASKV = -1536.0                  # band mask in PSUM units: SC*MASKV = -60
MASKD = -40000.0                 # diag mask added to master (x' < 0 -> 0)
W0 = 1024            # DVE width in c0 (covers all diag subtiles)
W2 = 256             # DVE width in c1

F32 = mybir.dt.float32
BF16 = mybir.dt.bfloat16
FP8E4 = mybir.dt.float8e4
FP8E5 = mybir.dt.float8e5
I16 = mybir.dt.int16
AF = mybir.ActivationFunctionType
ALU = mybir.AluOpType
DR = mybir.MatmulPerfMode.DoubleRow

_nc_cache = None


def _build():
    nc = bacc.Bacc(None, target_bir_lowering=False)
    zt = nc.dram_tensor("zt", [P, 2, WCOL], FP8E4, kind="ExternalInput")
    mtri_in = nc.dram_tensor("mtri", [P, P], F32, kind="ExternalInput")
    maskd_in = nc.dram_tensor("maskd", [P, P], F32, kind="ExternalInput")
    out_acc = nc.dram_tensor("out_acc", [P, IT * 6], F32, kind="ExternalOutput")
    out_cs = nc.dram_tensor("out_cs", [1, 4096], BF16, kind="ExternalOutput")

    with (
        tile.TileContext(nc) as tc,
        tc.tile_pool(name="sb", bufs=1) as sb,
        tc.tile_pool(name="big", bufs=3, space="PSUM") as bigp,
        tc.tile_pool(name="csp", bufs=1, space="PSUM") as cspp,
    ):
        zts = sb.tile([P, 2, WCOL], FP8E4)
        Ed = sb.tile([P, IT, W0 + W2], BF16)   # DVE E (schraudolph bits)
        E0a = sb.tile([P, IT, 2048 - W0], FP8E5)
        E1a = sb.tile([P, IT, 2048 - W2], FP8E5)
        Eb = sb.tile([P, IT, 1024], FP8E5)
        acc = sb.tile([P, IT, 6], F32)
        cs_sb = sb.tile([P, 4096], BF16)
        master = sb.tile([P, 2048], F32)
        mtri = sb.tile([P, P], F32)
        maskd = sb.tile([P, P], F32)
        ones2 = sb.tile([P, 2, P], FP8E5)      # DoubleRow colsum lhsT
        ones1b = sb.tile([P, P], BF16)         # plain bf16 colsum lhsT
        ones1 = sb.tile([P, P], FP8E5)         # plain fp8 colsum lhsT

        # input DMAs, split across queues so compute can start early
        nc.sync.dma_start(out=zts[:, :, 0:1024], in_=zt[:, :, 0:1024])
        nc.sync.dma_start(out=zts[:, :, 1024:2048], in_=zt[:, :, 1024:2048])
        nc.gpsimd.dma_start(out=zts[:, :, 2048:4096], in_=zt[:, :, 2048:4096])
        nc.scalar.dma_start(out=zts[:, :, 4096:WCOL], in_=zt[:, :, 4096:WCOL])
        nc.sync.dma_start(out=mtri, in_=mtri_in[:, :])
        nc.sync.dma_start(out=maskd, in_=maskd_in[:, :])

        nc.vector.memset(ones2, 1.0)
        nc.vector.memset(ones1b, 1.0)
        nc.vector.memset(ones1, 1.0)
        nc.vector.memset(acc, 0.0)
        # master: S2D everywhere; diag-mask block at cols [1024, 1152)
        # (row tile i's STT window [1024-128i, ...) puts the block exactly
        # over its self-sim diagonal subtile)
        nc.vector.memset(master, float(S2D))
        nc.vector.tensor_add(master[:, 1024:1152], master[:, 1024:1152], maskd)

        # colsum accumulators, held for the whole kernel
        cps1 = cspp.tile([P, 512], F32, name="cps1")
        cps2 = cspp.tile([P, 512], F32, name="cps2")

        def lhsT(i):
            return zts[:, :, i * P : (i + 1) * P]

        def emit_main(i):
            # ---- c0: cols [0, 2048) ----
            a0 = bigp.tile([P, W0], F32, tag="ps", name=f"a0_{i}")
            a1 = bigp.tile([P, 2048 - W0], F32, tag="ps", name=f"a1_{i}")
            for n in range(0, W0, 512):
                nc.tensor.matmul(
                    a0[:, n : n + 512], lhsT=lhsT(i),
                    rhs=zts[:, :, n : n + 512],
                    start=True, stop=True, perf_mode=DR,
                )
            for n in range(W0, 2048, 512):
                nc.tensor.matmul(
                    a1[:, n - W0 : n - W0 + 512], lhsT=lhsT(i),
                    rhs=zts[:, :, n : n + 512],
                    start=True, stop=True, perf_mode=DR,
                )
            # DVE: bf16 schraudolph with fused diag mask via master window
            nc.vector.scalar_tensor_tensor(
                out=Ed[:, i, 0:W0].bitcast(I16), in0=a0[:, :],
                scalar=float(S1D),
                in1=master[:, 1024 - P * i : 1024 - P * i + W0],
                op0=ALU.mult, op1=ALU.add,
            )
            # ACT: exp of [W0, 2048)
            nc.scalar.activation(
                out=E0a[:, i, :], in_=a1[:, :], func=AF.Exp, scale=float(SC),
                accum_out=acc[:, i, 1:2],
            )

            # ---- c1: cols [2048, 4096) ----
            b0 = bigp.tile([P, 1024], F32, tag="ps", name=f"b0_{i}")
            b1 = bigp.tile([P, 1024], F32, tag="ps", name=f"b1_{i}")
            for n in range(0, 1024, 512):
                nc.tensor.matmul(
                    b0[:, n : n + 512], lhsT=lhsT(i),
                    rhs=zts[:, :, 2048 + n : 2048 + n + 512],
                    start=True, stop=True, perf_mode=DR,
                )
            for n in range(0, 1024, 512):
                nc.tensor.matmul(
                    b1[:, n : n + 512], lhsT=lhsT(i),
                    rhs=zts[:, :, 3072 + n : 3072 + n + 512],
                    start=True, stop=True, perf_mode=DR,
                )
            nc.vector.scalar_tensor_tensor(
                out=Ed[:, i, W0 : W0 + W2].bitcast(I16), in0=b0[:, 0:W2],
                scalar=float(S1D), in1=master[:, 0:W2],
                op0=ALU.mult, op1=ALU.add,
            )
            # one reduce covers both DVE windows (c0 + c1)
            nc.vector.reduce_sum(
                acc[:, i, 0:1], Ed[:, i, :], axis=mybir.AxisListType.X
            )
            nc.scalar.activation(
                out=E1a[:, i, 0 : 1024 - W2], in_=b0[:, W2:1024],
                func=AF.Exp, scale=float(SC), accum_out=acc[:, i, 3:4],
            )
            nc.scalar.activation(
                out=E1a[:, i, 1024 - W2 : 2048 - W2], in_=b1[:, :],
                func=AF.Exp, scale=float(SC), accum_out=acc[:, i, 4:5],
            )

            # ---- band: cols [4096, 4096 + 128(i+1)) ----
            wb = P * (i + 1)
            cb = bigp.tile([P, 1024], F32, tag="ps", name=f"cb_{i}")
            full = wb - P
            n = 0
            while n < full:
                w = min(512 - (n % 512), full - n)
                nc.tensor.matmul(
                    cb[:, n : n + w], lhsT=lhsT(i),
                    rhs=zts[:, :, 4096 + n : 4096 + n + w],
                    start=True, stop=True, perf_mode=DR,
                )
                n += w
            nc.tensor.matmul(
                cb[:, full : full + P], lhsT=lhsT(i),
                rhs=zts[:, :, 4096 + full : 4096 + full + P],
                start=True, stop=True, perf_mode=DR,
            )
            nc.vector.tensor_add(
                cb[:, full : full + P], cb[:, full : full + P], mtri
            )
            nc.scalar.activation(
                out=Eb[:, i, 0:wb], in_=cb[:, 0:wb], func=AF.Exp,
                scale=float(SC), accum_out=acc[:, i, 5:6],
            )

        def emit_cs_pair(k):
            # interleaved chain segments for chunks ch0/ch1 = cols
            # [1024, 2048), row-tile pair (2k, 2k+1), full 128 partitions
            st = k == 0
            sp = k == 3
            nc.tensor.matmul(
                cps1[:, :], lhsT=ones2,
                rhs=E0a[:, 2 * k : 2 * k + 2, 0:512],
                start=st, stop=sp, perf_mode=DR,
            )
            nc.tensor.matmul(
                cps2[:, :], lhsT=ones2,
                rhs=E0a[:, 2 * k : 2 * k + 2, 512:1024],
                start=st, stop=sp, perf_mode=DR,
            )

        def chain_dr(dst, src, off, w):
            for k in range(4):
                nc.tensor.matmul(
                    dst, lhsT=ones2,
                    rhs=src[:, 2 * k : 2 * k + 2, off : off + w],
                    start=(k == 0), stop=(k == 3), perf_mode=DR,
                )

        def chain_bf(dst, src, off, w):
            for k in range(IT):
                nc.tensor.matmul(
                    dst, lhsT=ones1b,
                    rhs=src[:, k, off : off + w],
                    start=(k == 0), stop=(k == IT - 1),
                )

        def chain_band(dst, j):
            # band col chunk [4096+128j, +128): row tiles i >= j
            rts = list(range(j, IT))
            first = True
            x = 0
            while x + 1 < len(rts):
                nc.tensor.matmul(
                    dst, lhsT=ones2,
                    rhs=Eb[:, rts[x] : rts[x] + 2, 128 * j : 128 * (j + 1)],
                    start=first, stop=(x + 2 >= len(rts)), perf_mode=DR,
                )
                first = False
                x += 2
            if x < len(rts):
                nc.tensor.matmul(
                    dst, lhsT=ones1,
                    rhs=Eb[:, rts[x], 128 * j : 128 * (j + 1)],
                    start=first, stop=True,
                )

        def cs_copy(cpsrc, off, w, engine):
            # cs_sb[:, off:off+w] <- cps bank cols [0, w), bf16
            dst = cs_sb[:, off : off + w]
            if engine == "v":
                nc.vector.tensor_copy(out=dst, in_=cpsrc[:, 0:w])
            else:
                nc.scalar.copy(dst, cpsrc[:, 0:w])

        def emit_cs_tail():
            # cs_sb col x = colsum of local col 1024 + x
            cs_copy(cps1, 0, 512, "v")                # ch0  [1024, 1536)
            cs_copy(cps2, 512, 512, "s")              # ch1  [1536, 2048)
            chain_bf(cps1[:, 0:W2], Ed, W0, W2)       # ch2a [2048, 2304)
            chain_dr(cps2[:, :], E1a, 0, 512)         # ch2b [2304, 2816)
            cs_copy(cps1, 1024, W2, "v")
            chain_dr(cps1[:, :], E1a, 512, 512)       # ch3  [2816, 3328)
            cs_copy(cps2, 1024 + W2, 512, "s")
            chain_dr(cps2[:, :], E1a, 1024, 512)      # ch4  [3328, 3840)
            cs_copy(cps1, 1536 + W2, 512, "v")
            chain_dr(cps1[:, 0 : 512 - W2], E1a, 1536, 512 - W2)
            cs_copy(cps2, 2048 + W2, 512, "s")        # ch5  [3840, 4096)
            for j in range(4):                        # band A: [4096, 4608)
                chain_band(cps2[:, 128 * j : 128 * (j + 1)], j)
            cs_copy(cps1, 2816, 512 - W2, "v")
            for j in range(4, IT):                    # band B: [4608, 5120)
                chain_band(cps1[:, 128 * (j - 4) : 128 * (j - 3)], j)
            cs_copy(cps2, 3072, 512, "s")
            cs_copy(cps1, 3584, 512, "v")

        # ---- main schedule ----
        for i in range(IT):
            emit_main(i)
            if i % 2 == 1:
                emit_cs_pair(i // 2)
        emit_cs_tail()
        nc.sync.dma_start(
            out=out_acc[:, :], in_=acc.rearrange("p i s -> p (i s)")
        )
        nc.sync.dma_start(out=out_cs[0:1, :], in_=cs_sb[0:1, :])

    nc.finalize()
    return nc


def _get_nc():
    global _nc_cache
    if _nc_cache is None:
        _nc_cache = _build()
    return _nc_cache


def _prep_inputs(z: np.ndarray):
    """z: [M, D] float32 (unnormalized). Returns per-core input maps and
    host-side positive-pair cosines."""
    nrm = np.sqrt((z.astype(np.float64) ** 2).sum(axis=1))
    zn = z / np.maximum(nrm, 1e-8).astype(np.float32)[:, None]
    zn64 = zn.astype(np.float64)
    pos = (zn64 * np.roll(zn64, -N2, axis=0)).sum(axis=1)
    q = (QS * zn).astype(ml_dtypes.float8_e4m3)
    # [128, 2, 8192]: [p, h, col] = q[col, 128h + p]
    big = np.ascontiguousarray(q.T.reshape(2, P, M).transpose(1, 0, 2))
    mtri = np.where(
        np.arange(P)[None, :] >= np.arange(P)[:, None], MASKV, 0.0
    ).astype(np.float32)
    maskd = (MASKD * np.eye(P)).astype(np.float32)
    in_maps = []
    for c in range(NCORES):
        zr = np.roll(big, -1024 * c, axis=2)[:, :, :WCOL]
        in_maps.append(
            {"zt": np.ascontiguousarray(zr), "mtri": mtri, "maskd": maskd}
        )
    return in_maps, pos


def _run_cores(z: np.ndarray, trace: bool = False):
    from concourse.bass_utils import run_bass_kernel_spmd

    nc = _get_nc()
    in_maps, _ = _prep_inputs(np.asarray(z, np.float32))
    return run_bass_kernel_spmd(
        nc, in_maps, core_ids=list(range(NCORES)), trace=trace
    )


def _combine(results, pos):
    total = np.zeros(M, np.float64)
    for c, r in enumerate(results):
        accv = np.asarray(r["out_acc"]).astype(np.float64).reshape(P, IT, 6)
        cs = np.asarray(r["out_cs"]).astype(np.float64)[0]
        base = 1024 * c
        rows = accv.sum(axis=2)  # [P, IT]
        for i in range(IT):
            g = (base + i * P + np.arange(P)) % M
            total[g] += rows[:, i]
        # colsums for local cols [1024, 5120), in order
        total[(base + 1024 + np.arange(4096)) % M] += cs

    total += np.exp(TEMP_INV * pos)
    lse = np.log(total)
    return np.float32((lse - TEMP_INV * pos).mean())


def kernel(z1: np.ndarray, z2: np.ndarray) -> np.ndarray:
    from concourse.bass_utils import run_bass_kernel_spmd

    z = np.concatenate(
        [np.asarray(z1, np.float32), np.asarray(z2, np.float32)], axis=0
    )
    nc = _get_nc()
    in_maps, pos = _prep_inputs(z)
    res = run_bass_kernel_spmd(nc, in_maps, core_ids=list(range(NCORES)))
    return _combine(res.results, pos)
